# revision 7
# baseline (speedup 1.0000x reference)
"""ANI per-element MLP (MoE-routed) Trainium2 kernel, 8 NeuronCores.

Strategy
--------
The reference computes all 7 element MLPs for every atom and selects by
species (7x redundant).  Here the host routes instead: atoms are
stable-sorted by species, each species' atoms are split evenly across the
8 cores (identical padded group sizes G_s on every core, so one NEFF runs
SPMD), and each core runs its 7 dense per-species MLPs over contiguous
column groups.  No cross-core communication is needed (output is
per-atom).

Layout: activations are feature-major ([features(partition), atoms(free)])
so layers chain with zero transposes; the host supplies aev pre-transposed
as xT [1008, M] in bfloat16.  Matmuls run bf16 x bf16 -> fp32 PSUM
(full-rate); biases/activations apply in fp32 on ScalarE.

CELU folding: with a = celu(z)/alpha + 1 the device only needs
  e = Exp(u/a + c);  r = Relu(u/a + c);  a = min(e,1) + r
(2 ScalarE ops + 1 fused VectorE scalar_tensor_tensor), with the alpha
factors folded into the next layer's weights (W' = alpha*W) and biases
(b' = b - alpha*colsum(W)) on the host.  The final-layer bias is added on
the host during unpermute.
"""

import os
import sys

import ml_dtypes
import numpy as np

BF16 = ml_dtypes.bfloat16

for _p in ("/opt/trn_rl_repo", "/root/.axon_site/_ro/trn_rl_repo"):
    if os.path.isdir(_p) and _p not in sys.path:
        sys.path.insert(0, _p)

import concourse.bass as bass
import concourse.bacc as bacc
import concourse.mybir as mybir
from concourse.bass_utils import run_bass_kernel_spmd
from concourse.tile import TileContext

N_CORES = 8
AEV_DIM = 1008
ALPHA = 0.1
INV_ALPHA = 10.0
NSPEC = 7
# hidden sizes per element net: H, C, N, O, S, F, Cl
LAYERS = [
    (256, 192, 160),
    (224, 192, 160),
    (192, 160, 128),
    (192, 160, 128),
    (160, 128, 96),
    (160, 128, 96),
    (160, 128, 96),
]
NTILE = 512  # atoms per matmul free-dim tile (fp32 moving-operand max)
KC = 126  # 1008 = 8 * 126 contraction chunks

# matmul dtype: "bf16" (half DMA traffic, ~0.7% rel err) or "f32r"
# (full fp32 storage, PE full-rate at N>=256, ~fp32-ish accuracy)
MM_DT = os.environ.get("ANI_MM_DT", "bf16")

LAST_RESULT = None
_CACHE: dict = {}


def _chunks(n):
    out = []
    while n > 0:
        c = min(128, n)
        out.append(c)
        n -= c
    return out


def _wblob_len():
    return sum(AEV_DIM * h1 + h1 * h2 + h2 * h3 + h3 for h1, h2, h3 in LAYERS)


def _cblob_len():
    return sum(h1 + h2 + h3 for h1, h2, h3 in LAYERS)


def _build(G):
    """Build the SPMD Bass program for per-core species group sizes G."""
    f32 = mybir.dt.float32
    bf16 = mybir.dt.bfloat16 if MM_DT == "bf16" else mybir.dt.float32r
    EXP = mybir.ActivationFunctionType.Exp
    RELU = mybir.ActivationFunctionType.Relu
    MIN = mybir.AluOpType.min
    ADD = mybir.AluOpType.add

    M = int(sum(G))
    nc = bacc.Bacc()
    xt_d = nc.declare_dram_parameter("xt", [AEV_DIM, M], bf16, isOutput=False)
    wb_d = nc.declare_dram_parameter("wb", [_wblob_len()], bf16, isOutput=False)
    cb_d = nc.declare_dram_parameter("cb", [_cblob_len()], f32, isOutput=False)
    out_d = nc.declare_dram_parameter("out", [M], f32, isOutput=True)
    xt_v = xt_d[:].rearrange("(k p) m -> p k m", p=KC)

    with TileContext(nc) as tc:
        with (
            tc.tile_pool(name="wp", bufs=1) as wp,
            tc.tile_pool(name="xp", bufs=3) as xp,
            tc.tile_pool(name="tp", bufs=3) as tp,
            tc.tile_pool(name="ab", bufs=2) as ab,
            tc.tile_pool(name="yp", bufs=1) as yp,
            tc.tile_pool(name="pp", bufs=2, space="PSUM") as pp,
        ):
            y = yp.tile([1, M], f32, name="yrow", tag="yrow")

            def load_weights(s, off, coff):
                h1, h2, h3 = LAYERS[s]
                ch1, ch2, ch3 = _chunks(h1), _chunks(h2), _chunks(h3)
                w1 = wp.tile([KC, 8, h1], bf16, name=f"w1s{s}", tag=f"w1s{s}")
                nc.sync.dma_start(
                    out=w1[:],
                    in_=wb_d[off : off + AEV_DIM * h1].rearrange(
                        "(k p h) -> p k h", p=KC, h=h1
                    ),
                )
                off += AEV_DIM * h1
                w2 = wp.tile([128, len(ch1), h2], bf16, name=f"w2s{s}", tag=f"w2s{s}")
                for ci, cs in enumerate(ch1):
                    nc.sync.dma_start(
                        out=w2[0:cs, ci, :],
                        in_=wb_d[off : off + cs * h2].rearrange("(p h) -> p h", h=h2),
                    )
                    off += cs * h2
                w3 = wp.tile([128, len(ch2), h3], bf16, name=f"w3s{s}", tag=f"w3s{s}")
                for ci, cs in enumerate(ch2):
                    nc.sync.dma_start(
                        out=w3[0:cs, ci, :],
                        in_=wb_d[off : off + cs * h3].rearrange("(p h) -> p h", h=h3),
                    )
                    off += cs * h3
                w4 = wp.tile([128, len(ch3), 1], bf16, name=f"w4s{s}", tag=f"w4s{s}")
                for ci, cs in enumerate(ch3):
                    nc.sync.dma_start(
                        out=w4[0:cs, ci, :],
                        in_=wb_d[off : off + cs].rearrange("(p h) -> p h", h=1),
                    )
                    off += cs
                cts = []
                for li, ch in ((1, ch1), (2, ch2), (3, ch3)):
                    ct = wp.tile([128, len(ch)], f32, name=f"c{li}s{s}", tag=f"c{li}s{s}")
                    for ci, cs in enumerate(ch):
                        nc.sync.dma_start(
                            out=ct[0:cs, ci : ci + 1],
                            in_=cb_d[coff : coff + cs].rearrange("(p h) -> p h", h=1),
                        )
                        coff += cs
                    cts.append(ct)
                return (w1, w2, w3, w4, *cts), off, coff

            def layer(rhs, kch, mch, w, ct, nt, tagp):
                """rhs: list of k-chunk APs; returns list of activation tiles."""
                outs = []
                nk = len(kch)
                for mi, mc in enumerate(mch):
                    ps = pp.tile([128, NTILE], f32, name=f"ps{tagp}", tag=f"ps{tagp}")
                    for ki, kc in enumerate(kch):
                        nc.tensor.matmul(
                            ps[0:mc, 0:nt],
                            lhsT=w[0:kc, ki, mi * 128 : mi * 128 + mc],
                            rhs=rhs[ki][0:kc, 0:nt],
                            start=(ki == 0),
                            stop=(ki == nk - 1),
                        )
                    e = tp.tile([128, NTILE], bf16, name="et", tag="et")
                    r = tp.tile([128, NTILE], bf16, name="rt", tag="rt")
                    nc.scalar.activation(
                        e[0:mc, 0:nt], ps[0:mc, 0:nt], EXP,
                        bias=ct[0:mc, mi : mi + 1], scale=INV_ALPHA,
                    )
                    nc.scalar.activation(
                        r[0:mc, 0:nt], ps[0:mc, 0:nt], RELU,
                        bias=ct[0:mc, mi : mi + 1], scale=INV_ALPHA,
                    )
                    am = ab.tile([128, NTILE], bf16, name=f"a{tagp}{mi}", tag=f"a{tagp}{mi}")
                    nc.vector.scalar_tensor_tensor(
                        am[0:mc, 0:nt], e[0:mc, 0:nt], 1.0, r[0:mc, 0:nt], MIN, ADD
                    )
                    outs.append(am)
                return outs

            off = 0
            coff = 0
            col0 = 0
            for s in range(NSPEC):
                h1, h2, h3 = LAYERS[s]
                ch1, ch2, ch3 = _chunks(h1), _chunks(h2), _chunks(h3)
                wt, off, coff = load_weights(s, off, coff)
                w1, w2, w3, w4, c1, c2, c3 = wt
                for t0 in range(0, G[s], NTILE):
                    nt = min(NTILE, G[s] - t0)
                    a0 = col0 + t0
                    xt = xp.tile([KC, 8, NTILE], bf16, name="xtile", tag="xtile")
                    nc.sync.dma_start(out=xt[:, :, 0:nt], in_=xt_v[:, :, a0 : a0 + nt])
                    a1 = layer([xt[:, k, :] for k in range(8)], [KC] * 8, ch1, w1, c1, nt, "1")
                    a2 = layer(a1, ch1, ch2, w2, c2, nt, "2")
                    a3 = layer(a2, ch2, ch3, w3, c3, nt, "3")
                    p4 = pp.tile([1, NTILE], f32, name="p4", tag="p4")
                    for ki, kc in enumerate(ch3):
                        nc.tensor.matmul(
                            p4[0:1, 0:nt],
                            lhsT=w4[0:kc, ki, 0:1],
                            rhs=a3[ki][0:kc, 0:nt],
                            start=(ki == 0),
                            stop=(ki == len(ch3) - 1),
                        )
                    nc.any.tensor_copy(y[0:1, a0 : a0 + nt], p4[0:1, 0:nt])
                col0 += G[s]
            assert off == _wblob_len() and coff == _cblob_len()
            nc.sync.dma_start(out=out_d[:].rearrange("(o m) -> o m", o=1), in_=y[0:1, :])
    nc.finalize()
    return nc


def kernel(species, aev, params):
    global LAST_RESULT
    species = np.asarray(species).astype(np.int64)
    aev = np.asarray(aev, dtype=np.float32)
    n_atoms = species.shape[0]

    # ---- fold CELU alpha into weights/biases ----
    wparts, cparts = [], []
    b4p = np.zeros(NSPEC, np.float32)
    for s in range(NSPEC):
        W1, b1, W2, b2, W3, b3, W4, b4 = [np.asarray(t, dtype=np.float32) for t in params[s]]
        wparts += [
            W1.ravel(),
            (ALPHA * W2).ravel(),
            (ALPHA * W3).ravel(),
            (ALPHA * W4).ravel(),
        ]
        cparts += [
            INV_ALPHA * b1,
            INV_ALPHA * b2 - W2.sum(axis=0),
            INV_ALPHA * b3 - W3.sum(axis=0),
        ]
        b4p[s] = b4[0] - ALPHA * W4.sum()
    mmdt = BF16 if MM_DT == "bf16" else np.float32
    wblob = np.ascontiguousarray(np.concatenate(wparts).astype(mmdt))
    cblob = np.ascontiguousarray(np.concatenate(cparts), dtype=np.float32)
    assert wblob.shape[0] == _wblob_len() and cblob.shape[0] == _cblob_len()

    # ---- route: stable-sort by species, split each species evenly over cores ----
    counts = np.bincount(species, minlength=NSPEC).astype(np.int64)
    # even group sizes: fp32r matmuls require an even moving free dim
    G = tuple((g + (g & 1)) for g in (int(-(-int(c) // N_CORES)) for c in counts))
    M = int(sum(G))
    order = np.argsort(species, kind="stable")
    sofs = np.concatenate([[0], np.cumsum(counts)])
    core_parts = [[] for _ in range(N_CORES)]  # (species, src_indices) per group
    for s in range(NSPEC):
        if counts[s] == 0:
            continue
        block = order[sofs[s] : sofs[s + 1]]
        q, r = divmod(int(counts[s]), N_CORES)
        pos = 0
        for c in range(N_CORES):
            n = q + (1 if c < r else 0)
            core_parts[c].append((s, block[pos : pos + n]))
            pos += n

    in_maps = []
    for c in range(N_CORES):
        Xc = np.zeros((M, AEV_DIM), mmdt)
        col = 0
        for s, ch in core_parts[c]:
            Xc[col : col + len(ch)] = aev[ch]
            col += G[s]
        xT = np.ascontiguousarray(Xc.T)
        in_maps.append({"xt": xT, "wb": wblob, "cb": cblob})

    # ---- build (cached), run ----
    ck = (G, MM_DT)
    if ck not in _CACHE:
        _CACHE[ck] = _build(G)
    nc = _CACHE[ck]
    trace = os.environ.get("BASS_KERNEL_TRACE", "0") == "1"
    res = run_bass_kernel_spmd(nc, in_maps, core_ids=list(range(N_CORES)), trace=trace)
    LAST_RESULT = res

    # ---- unpermute + final-layer bias ----
    out = np.zeros(n_atoms, np.float32)
    for c in range(N_CORES):
        yc = np.asarray(res.results[c]["out"], dtype=np.float32)
        col = 0
        for s, ch in core_parts[c]:
            out[ch] = yc[col : col + len(ch)] + b4p[s]
            col += G[s]
    return out


# revision 8
# speedup vs baseline: 1.4003x; 1.4003x over previous
"""ANI per-element MLP (MoE-routed) Trainium2 kernel, 8 NeuronCores.

Strategy
--------
The reference computes all 7 element MLPs for every atom and selects by
species (7x redundant).  Here the host routes instead: atoms are
stable-sorted by species, each species' atoms are split evenly across the
8 cores (identical padded group sizes G_s on every core, so one NEFF runs
SPMD), and each core runs its 7 dense per-species MLPs over contiguous
column groups.  No cross-core communication is needed (output is
per-atom).

Layout: activations are feature-major ([features(partition), atoms(free)])
so layers chain with zero transposes; the host supplies aev pre-transposed
as xT [1008, M] in bfloat16.  Matmuls run bf16 x bf16 -> fp32 PSUM
(full-rate); biases/activations apply in fp32 on ScalarE.

CELU folding: with a = celu(z)/alpha + 1 the device only needs
  e = Exp(u/a + c);  r = Relu(u/a + c);  a = min(e,1) + r
(2 ScalarE ops + 1 fused VectorE scalar_tensor_tensor), with the alpha
factors folded into the next layer's weights (W' = alpha*W) and biases
(b' = b - alpha*colsum(W)) on the host.  The final-layer bias is added on
the host during unpermute.
"""

import os
import sys

import ml_dtypes
import numpy as np

BF16 = ml_dtypes.bfloat16

for _p in ("/opt/trn_rl_repo", "/root/.axon_site/_ro/trn_rl_repo"):
    if os.path.isdir(_p) and _p not in sys.path:
        sys.path.insert(0, _p)

import concourse.bass as bass
import concourse.bacc as bacc
import concourse.mybir as mybir
from concourse.bass_utils import run_bass_kernel_spmd
from concourse.tile import TileContext

N_CORES = 8
AEV_DIM = 1008
ALPHA = 0.1
INV_ALPHA = 10.0
NSPEC = 7
# hidden sizes per element net: H, C, N, O, S, F, Cl
LAYERS = [
    (256, 192, 160),
    (224, 192, 160),
    (192, 160, 128),
    (192, 160, 128),
    (160, 128, 96),
    (160, 128, 96),
    (160, 128, 96),
]
NTILE = 512  # atoms per matmul free-dim tile (fp32 moving-operand max)
KC = 126  # 1008 = 8 * 126 contraction chunks

# matmul dtype: "bf16" (half DMA traffic, ~0.7% rel err) or "f32r"
# (full fp32 storage, PE full-rate at N>=256, ~fp32-ish accuracy)
MM_DT = os.environ.get("ANI_MM_DT", "bf16")

LAST_RESULT = None
_CACHE: dict = {}


def _chunks(n):
    out = []
    while n > 0:
        c = min(128, n)
        out.append(c)
        n -= c
    return out


def _wblob_len():
    return sum(AEV_DIM * h1 + h1 * h2 + h2 * h3 + h3 for h1, h2, h3 in LAYERS)


def _cblob_len():
    return sum(h1 + h2 + h3 for h1, h2, h3 in LAYERS)


def _build(G):
    """Build the SPMD Bass program for per-core species group sizes G."""
    f32 = mybir.dt.float32
    bf16 = mybir.dt.bfloat16 if MM_DT == "bf16" else mybir.dt.float32r
    EXP = mybir.ActivationFunctionType.Exp
    RELU = mybir.ActivationFunctionType.Relu
    MIN = mybir.AluOpType.min
    ADD = mybir.AluOpType.add

    M = int(sum(G))
    nc = bacc.Bacc()
    xt_d = nc.declare_dram_parameter("xt", [AEV_DIM, M], bf16, isOutput=False)
    wb_d = nc.declare_dram_parameter("wb", [_wblob_len()], bf16, isOutput=False)
    cb_d = nc.declare_dram_parameter("cb", [_cblob_len()], f32, isOutput=False)
    out_d = nc.declare_dram_parameter("out", [M], f32, isOutput=True)
    xt_v = xt_d[:].rearrange("(k p) m -> p k m", p=KC)

    with TileContext(nc) as tc:
        with (
            tc.tile_pool(name="wp", bufs=1) as wp,
            tc.tile_pool(name="xp", bufs=3) as xp,
            tc.tile_pool(name="tp", bufs=3) as tp,
            tc.tile_pool(name="ab", bufs=3) as ab,
            tc.tile_pool(name="yp", bufs=1) as yp,
            tc.tile_pool(name="pp", bufs=2, space="PSUM") as pp,
        ):
            y = yp.tile([1, M], f32, name="yrow", tag="yrow")

            def load_weights(s, off, coff):
                h1, h2, h3 = LAYERS[s]
                ch1, ch2, ch3 = _chunks(h1), _chunks(h2), _chunks(h3)
                w1 = wp.tile([KC, 8, h1], bf16, name=f"w1s{s}", tag=f"w1s{s}")
                nc.sync.dma_start(
                    out=w1[:],
                    in_=wb_d[off : off + AEV_DIM * h1].rearrange(
                        "(k p h) -> p k h", p=KC, h=h1
                    ),
                )
                off += AEV_DIM * h1
                w2 = wp.tile([128, len(ch1), h2], bf16, name=f"w2s{s}", tag=f"w2s{s}")
                for ci, cs in enumerate(ch1):
                    nc.sync.dma_start(
                        out=w2[0:cs, ci, :],
                        in_=wb_d[off : off + cs * h2].rearrange("(p h) -> p h", h=h2),
                    )
                    off += cs * h2
                w3 = wp.tile([128, len(ch2), h3], bf16, name=f"w3s{s}", tag=f"w3s{s}")
                for ci, cs in enumerate(ch2):
                    nc.sync.dma_start(
                        out=w3[0:cs, ci, :],
                        in_=wb_d[off : off + cs * h3].rearrange("(p h) -> p h", h=h3),
                    )
                    off += cs * h3
                w4 = wp.tile([128, len(ch3), 1], bf16, name=f"w4s{s}", tag=f"w4s{s}")
                for ci, cs in enumerate(ch3):
                    nc.sync.dma_start(
                        out=w4[0:cs, ci, :],
                        in_=wb_d[off : off + cs].rearrange("(p h) -> p h", h=1),
                    )
                    off += cs
                cts = []
                for li, ch in ((1, ch1), (2, ch2), (3, ch3)):
                    ct = wp.tile([128, len(ch)], f32, name=f"c{li}s{s}", tag=f"c{li}s{s}")
                    for ci, cs in enumerate(ch):
                        nc.sync.dma_start(
                            out=ct[0:cs, ci : ci + 1],
                            in_=cb_d[coff : coff + cs].rearrange("(p h) -> p h", h=1),
                        )
                        coff += cs
                    cts.append(ct)
                return (w1, w2, w3, w4, *cts), off, coff

            def layer(rhs, kch, mch, w, ct, nt, tagp):
                """rhs: list of k-chunk APs; returns list of activation tiles."""
                outs = []
                nk = len(kch)
                for mi, mc in enumerate(mch):
                    ps = pp.tile([128, NTILE], f32, name=f"ps{tagp}", tag=f"ps{tagp}")
                    for ki, kc in enumerate(kch):
                        nc.tensor.matmul(
                            ps[0:mc, 0:nt],
                            lhsT=w[0:kc, ki, mi * 128 : mi * 128 + mc],
                            rhs=rhs[ki][0:kc, 0:nt],
                            start=(ki == 0),
                            stop=(ki == nk - 1),
                        )
                    e = tp.tile([128, NTILE], bf16, name="et", tag="et")
                    r = tp.tile([128, NTILE], bf16, name="rt", tag="rt")
                    nc.scalar.activation(
                        e[0:mc, 0:nt], ps[0:mc, 0:nt], EXP,
                        bias=ct[0:mc, mi : mi + 1], scale=INV_ALPHA,
                    )
                    nc.scalar.activation(
                        r[0:mc, 0:nt], ps[0:mc, 0:nt], RELU,
                        bias=ct[0:mc, mi : mi + 1], scale=INV_ALPHA,
                    )
                    am = ab.tile([128, NTILE], bf16, name=f"a{tagp}{mi}", tag=f"a{tagp}{mi}")
                    nc.vector.scalar_tensor_tensor(
                        am[0:mc, 0:nt], e[0:mc, 0:nt], 1.0, r[0:mc, 0:nt], MIN, ADD
                    )
                    outs.append(am)
                return outs

            # ---- flatten (species, tile) list; 4-stage skewed pipeline ----
            tiles = []  # (s, a0, nt, first_of_species)
            col0 = 0
            for s in range(NSPEC):
                for ti, t0 in enumerate(range(0, G[s], NTILE)):
                    tiles.append((s, col0 + t0, min(NTILE, G[s] - t0), ti == 0))
                col0 += G[s]

            off = 0
            coff = 0
            WT = {}
            state = [dict() for _ in tiles]

            def st_l1(i):
                nonlocal off, coff
                s, a0, nt, first = tiles[i]
                if first:
                    WT[s] = load_weights(s, off, coff)
                    _, off, coff = WT[s]
                (w1, w2, w3, w4, c1, c2, c3), _, _ = WT[s]
                ch1 = _chunks(LAYERS[s][0])
                xt = xp.tile([KC, 8, NTILE], bf16, name="xtile", tag="xtile")
                nc.sync.dma_start(out=xt[:, :, 0:nt], in_=xt_v[:, :, a0 : a0 + nt])
                state[i]["a1"] = layer([xt[:, k, :] for k in range(8)], [KC] * 8, ch1, w1, c1, nt, "1")

            def st_l2(i):
                s, a0, nt, _ = tiles[i]
                (w1, w2, w3, w4, c1, c2, c3), _, _ = WT[s]
                h1, h2, h3 = LAYERS[s]
                state[i]["a2"] = layer(state[i].pop("a1"), _chunks(h1), _chunks(h2), w2, c2, nt, "2")

            def st_l3(i):
                s, a0, nt, _ = tiles[i]
                (w1, w2, w3, w4, c1, c2, c3), _, _ = WT[s]
                h1, h2, h3 = LAYERS[s]
                state[i]["a3"] = layer(state[i].pop("a2"), _chunks(h2), _chunks(h3), w3, c3, nt, "3")

            def st_l4(i):
                s, a0, nt, _ = tiles[i]
                (w1, w2, w3, w4, c1, c2, c3), _, _ = WT[s]
                ch3 = _chunks(LAYERS[s][2])
                a3 = state[i].pop("a3")
                p4 = pp.tile([1, NTILE], f32, name="p4", tag="p4")
                for ki, kc in enumerate(ch3):
                    nc.tensor.matmul(
                        p4[0:1, 0:nt],
                        lhsT=w4[0:kc, ki, 0:1],
                        rhs=a3[ki][0:kc, 0:nt],
                        start=(ki == 0),
                        stop=(ki == len(ch3) - 1),
                    )
                nc.any.tensor_copy(y[0:1, a0 : a0 + nt], p4[0:1, 0:nt])

            stages = (st_l1, st_l2, st_l3, st_l4)
            nt_total = len(tiles)
            for it in range(nt_total + len(stages) - 1):
                for j, stg in enumerate(stages):
                    ti = it - j
                    if 0 <= ti < nt_total:
                        stg(ti)
            assert off == _wblob_len() and coff == _cblob_len()
            nc.sync.dma_start(out=out_d[:].rearrange("(o m) -> o m", o=1), in_=y[0:1, :])
    nc.finalize()
    return nc


def kernel(species, aev, params):
    global LAST_RESULT
    species = np.asarray(species).astype(np.int64)
    aev = np.asarray(aev, dtype=np.float32)
    n_atoms = species.shape[0]

    # ---- fold CELU alpha into weights/biases ----
    wparts, cparts = [], []
    b4p = np.zeros(NSPEC, np.float32)
    for s in range(NSPEC):
        W1, b1, W2, b2, W3, b3, W4, b4 = [np.asarray(t, dtype=np.float32) for t in params[s]]
        wparts += [
            W1.ravel(),
            (ALPHA * W2).ravel(),
            (ALPHA * W3).ravel(),
            (ALPHA * W4).ravel(),
        ]
        cparts += [
            INV_ALPHA * b1,
            INV_ALPHA * b2 - W2.sum(axis=0),
            INV_ALPHA * b3 - W3.sum(axis=0),
        ]
        b4p[s] = b4[0] - ALPHA * W4.sum()
    mmdt = BF16 if MM_DT == "bf16" else np.float32
    wblob = np.ascontiguousarray(np.concatenate(wparts).astype(mmdt))
    cblob = np.ascontiguousarray(np.concatenate(cparts), dtype=np.float32)
    assert wblob.shape[0] == _wblob_len() and cblob.shape[0] == _cblob_len()

    # ---- route: stable-sort by species, split each species evenly over cores ----
    counts = np.bincount(species, minlength=NSPEC).astype(np.int64)
    # even group sizes: fp32r matmuls require an even moving free dim
    G = tuple((g + (g & 1)) for g in (int(-(-int(c) // N_CORES)) for c in counts))
    M = int(sum(G))
    order = np.argsort(species, kind="stable")
    sofs = np.concatenate([[0], np.cumsum(counts)])
    core_parts = [[] for _ in range(N_CORES)]  # (species, src_indices) per group
    for s in range(NSPEC):
        if counts[s] == 0:
            continue
        block = order[sofs[s] : sofs[s + 1]]
        q, r = divmod(int(counts[s]), N_CORES)
        pos = 0
        for c in range(N_CORES):
            n = q + (1 if c < r else 0)
            core_parts[c].append((s, block[pos : pos + n]))
            pos += n

    in_maps = []
    for c in range(N_CORES):
        Xc = np.zeros((M, AEV_DIM), mmdt)
        col = 0
        for s, ch in core_parts[c]:
            Xc[col : col + len(ch)] = aev[ch]
            col += G[s]
        xT = np.ascontiguousarray(Xc.T)
        in_maps.append({"xt": xT, "wb": wblob, "cb": cblob})

    # ---- build (cached), run ----
    ck = (G, MM_DT)
    if ck not in _CACHE:
        _CACHE[ck] = _build(G)
    nc = _CACHE[ck]
    trace = os.environ.get("BASS_KERNEL_TRACE", "0") == "1"
    res = run_bass_kernel_spmd(nc, in_maps, core_ids=list(range(N_CORES)), trace=trace)
    LAST_RESULT = res

    # ---- unpermute + final-layer bias ----
    out = np.zeros(n_atoms, np.float32)
    for c in range(N_CORES):
        yc = np.asarray(res.results[c]["out"], dtype=np.float32)
        col = 0
        for s, ch in core_parts[c]:
            out[ch] = yc[col : col + len(ch)] + b4p[s]
            col += G[s]
    return out
